# revision 1
# baseline (speedup 1.0000x reference)
"""MultiHeadSelection Trainium2 kernel.

scores[b,i,j,p] = sum_k tanh(x[b,i]@u_a[:,k] + x[b,j]@w_a[:,k] + b_s[k]) * v[k,p]

Shapes (hardcoded): x [8,256,768], u_a/w_a [768,256], b_s [256], v [256,50]
-> out [8,256,256,50] float32.

Sharding: data-parallel over batch, one batch element per NeuronCore (8 cores).
Each core:
  stage 1 (tiny): left_T[k,i] = (x_b @ u_a)^T, right_T[k,j] = (x_b @ w_a)^T
                  via PE matmuls with k on psum partitions (lhsT = weights
                  [h,k] chunk, rhs = x^T [h,*]); bias_all = left_T + b_s.
  stage 2 (hot):  for each i: pre[k,j] = right_T[k,j] + bias_all[k,i]
                  (DVE tensor_scalar, bf16 4x mode), tanh on ACT in big
                  FD=IB*256 ops, then PE matmuls lhsT=tanh[k, j-chunk]
                  (128-col bf16 weights -> FWL), rhs = v[k-chunk] bf16,
                  accumulated over the 2 k-chunks into psum [j_local, p].
                  psum -> SBUF staging (DVE) -> one 819KB DMA per 16-i block.
"""

import numpy as np
from contextlib import ExitStack

import concourse.bass as bass
import concourse.mybir as mybir
import concourse.tile as tile
from concourse import bacc

B, S, H, K, P = 8, 256, 768, 256, 50
NCORES = 8
IB = 16            # i-block size (ACT op free dim = IB*S = 4096)
GRP = 4            # i's per psum output tile ([128, GRP*2*50] = 1600B/bank)
KC = K // 128      # 2 k-chunks
HC = H // 128      # 6 h-chunks
JC = S // 128      # 2 j-chunks

F32 = mybir.dt.float32
BF16 = mybir.dt.bfloat16


def _build_nc(reps=1, ablate=()):
    ablate = set(ablate)
    # reps>1 repeats the whole computation on-device (same inputs/outputs) —
    # used only for timing: wall(R=3) - wall(R=1) isolates device time from
    # the per-call host/axon dispatch overhead.
    # Bacc (not raw Bass): its compile() pass splits multi-semaphore waits
    # into EventSemaphore instructions — TRN2 engine instructions hold 1 wait.
    nc = bacc.Bacc("TRN2", target_bir_lowering=False, debug=False,
                   enable_partition_id=False)

    xb = nc.dram_tensor("xb", [S, H], F32, kind="ExternalInput").ap()
    ua = nc.dram_tensor("ua", [H, K], F32, kind="ExternalInput").ap()
    wa = nc.dram_tensor("wa", [H, K], F32, kind="ExternalInput").ap()
    bs = nc.dram_tensor("bs", [K], F32, kind="ExternalInput").ap()
    vv = nc.dram_tensor("vv", [K, P], F32, kind="ExternalInput").ap()
    sc = nc.dram_tensor("scores", [S, S, P], F32, kind="ExternalOutput").ap()

    with ExitStack() as ctx:
        tc = ctx.enter_context(tile.TileContext(nc))
        singles = ctx.enter_context(tc.tile_pool(name="singles", bufs=1))
        work = ctx.enter_context(tc.tile_pool(name="work", bufs=2))
        outp = ctx.enter_context(tc.tile_pool(name="outp", bufs=2))

        # ---- constants ----
        v_bf = singles.tile([128, KC, P], BF16)
        for kc in range(KC):
            nc.gpsimd.dma_start(out=v_bf[:, kc, :], in_=vv[kc * 128:(kc + 1) * 128, :])
        bs_dma = singles.tile([128, KC], F32)
        for kc in range(KC):
            nc.sync.dma_start(out=bs_dma[:, kc:kc + 1], in_=bs[kc * 128:(kc + 1) * 128])
        # Bounce through a DVE copy so the DMA-completion wait lands on the
        # copy, not on the single-wait-slot TensorScalarPtr that consumes it.
        bs_col = singles.tile([128, KC], F32)
        nc.vector.tensor_copy(out=bs_col, in_=bs_dma)

        r_bf = singles.tile([128, KC, S], BF16)       # right_T, bf16
        bias_all = singles.tile([128, KC, S], F32)    # left_T + b_s, fp32

        # ---- stage 1 ----
        with tc.tile_pool(name="s1", bufs=1) as s1, \
             tc.tile_pool(name="s1d", bufs=1, space="DRAM") as s1d, \
             tc.tile_pool(name="ps1", bufs=2, space="PSUM") as ps1:
            u_bf = s1.tile([128, HC, K], BF16)
            w_bf = s1.tile([128, HC, K], BF16)
            for hc in range(HC):
                nc.gpsimd.dma_start(out=u_bf[:, hc, :], in_=ua[hc * 128:(hc + 1) * 128, :])
                nc.gpsimd.dma_start(out=w_bf[:, hc, :], in_=wa[hc * 128:(hc + 1) * 128, :])

            # x -> bf16 (DRAM scratch) -> transposed into SBUF as [h, i]
            xd = s1d.tile([S, H], BF16)
            nc.gpsimd.dma_start(out=xd, in_=xb)  # fp32 -> bf16 cast in DMA
            x_T = s1.tile([128, HC, S], BF16)
            for hc in range(HC):
                nc.sync.dma_start_transpose(out=x_T[:, hc, :], in_=xd[:, hc * 128:(hc + 1) * 128])

            for kc in range(KC):
                ps_r = ps1.tile([128, S], F32, tag="ps_r")
                ps_l = ps1.tile([128, S], F32, tag="ps_l")
                for hc in range(HC):
                    nc.tensor.matmul(ps_r, lhsT=w_bf[:, hc, kc * 128:(kc + 1) * 128],
                                     rhs=x_T[:, hc, :], start=(hc == 0), stop=(hc == HC - 1))
                for hc in range(HC):
                    nc.tensor.matmul(ps_l, lhsT=u_bf[:, hc, kc * 128:(kc + 1) * 128],
                                     rhs=x_T[:, hc, :], start=(hc == 0), stop=(hc == HC - 1))
                nc.vector.tensor_copy(out=r_bf[:, kc, :], in_=ps_r)
                # Two-step (copy then add) keeps the TensorScalarPtr at a
                # single semaphore wait: its ISA encoding has only one wait
                # slot, and a direct PSUM read would need PE + DMA waits.
                lt = s1.tile([128, S], F32, tag="lt")
                nc.vector.tensor_copy(out=lt, in_=ps_l)
                nc.vector.tensor_scalar_add(out=bias_all[:, kc, :], in0=lt,
                                            scalar1=bs_col[:, kc:kc + 1])

        # ---- stage 2 ----
        pso = ctx.enter_context(tc.tile_pool(name="pso", bufs=6, space="PSUM"))
        lin_scr = None
        if "lin_dma" in ablate or "relay" in ablate or "relay2" in ablate:
            lind = ctx.enter_context(tc.tile_pool(name="lind", bufs=1, space="DRAM"))
            lin_scr = lind.tile([S // IB, 128, IB, JC, P], F32)
        for blk in [b for _ in range(reps) for b in range(S // IB)]:
            pre = work.tile([128, KC, IB, S], BF16, tag="pre")
            th = work.tile([128, KC, IB, S], BF16, tag="th")
            # Absorb the buffer-reuse (WAR vs ACT) semaphore waits into this
            # memset: the TensorScalarPtr ISA struct has only one sync-wait
            # slot, so the preadds below must not carry cross-engine waits.
            nc.vector.memset(pre[:, 0, 0, 0:2], 0.0)
            for kc in range(KC):
                if "half_pre" in ablate:
                    for il in range(0, IB, 2):
                        i = blk * IB + il
                        # timing-only ablation: content is wrong, shape/rate match
                        nc.vector.tensor_scalar_add(out=pre[:, kc, il:il + 2, :],
                                                    in0=r_bf[:, 0:2, :],
                                                    scalar1=bias_all[:, kc, i:i + 1])
                else:
                    for il in range(IB):
                        i = blk * IB + il
                        nc.vector.tensor_scalar_add(out=pre[:, kc, il, :], in0=r_bf[:, kc, :],
                                                    scalar1=bias_all[:, kc, i:i + 1])
                if "no_act" not in ablate:
                    nc.scalar.activation(out=th[:, kc], in_=pre[:, kc],
                                         func=mybir.ActivationFunctionType.Tanh)
            src = pre if "no_act" in ablate else th
            kcs = [0] if "half_mm" in ablate else list(range(KC))
            if "v2" in ablate:
                # cols32 compute + 4-block output staging, stored in 8 large
                # DMAs per superblock: [32 part, 16 sb_g, 800B] each -> few
                # dma_starts (~1us fixed each) and 800B descriptor runs.
                nt, ng = 4, 4
                if blk % 4 == 0:
                    ost4 = outp.tile([128, 16, JC, nt, P], F32, tag="ost4")
                for g in range(IB // ng):
                    po = pso.tile([128, JC, nt, P], F32, tag="po")
                    for jc in range(JC):
                        for t in range(nt):
                            for kc in kcs:
                                for il_lo in range(ng):
                                    il = g * ng + il_lo
                                    wcols = src[:, kc, il, :].rearrange(
                                        "p (jc c t) -> p jc t c", jc=JC, t=nt)[:, jc, t, :]
                                    nc.tensor.matmul(
                                        po[32 * il_lo:32 * (il_lo + 1), jc, t, :],
                                        lhsT=wcols, rhs=v_bf[:, kc, :],
                                        start=(kc == kcs[0]), stop=(kc == kcs[-1]),
                                        tile_position=(0, 32 * il_lo))
                    nc.vector.tensor_copy(out=ost4[:, (blk % 4) * 4 + g], in_=po)
                if blk % 4 == 3:
                    i00 = (blk % (S // IB) - 3) * IB  # first i of the superblock
                    # out[c, sb_g, t, p] = scores[i00 + sb_g*4 + il_lo,
                    #                             jc*128 + 4c + t, p]
                    oap_all = sc[i00:i00 + 64].rearrange(
                        "(sbg il) (jc c t) p -> il c sbg jc (t p)", il=4, jc=JC, t=nt)
                    for il_lo in range(4):
                        for jc in range(JC):
                            nc.sync.dma_start(out=oap_all[il_lo, :, :, jc, :],
                                              in_=ost4[32 * il_lo:32 * (il_lo + 1), :, jc])
                continue
            nt = 4 if "cols32" in ablate else (2 if "cols64" in ablate else 0)
            if nt:
                # nt-way strided weight tiles (128//nt cols each), col-tiled
                # across psum groups: psum partition holds nt consecutive j ->
                # nt*200B-contiguous DRAM runs per descriptor.
                w = 128 // nt          # cols per tile
                ng = 128 // w          # psum col-groups per 128 partitions
                ost = outp.tile([128, IB // ng, JC, nt, P], F32, tag="ost")
                for g in range(IB // ng):
                    po = pso.tile([128, JC, nt, P], F32, tag="po")
                    for jc in range(JC):
                        for t in range(nt):
                            for kc in kcs:
                                for il_lo in range(ng):
                                    il = g * ng + il_lo
                                    wcols = src[:, kc, il, :].rearrange(
                                        "p (jc c t) -> p jc t c", jc=JC, t=nt)[:, jc, t, :]
                                    nc.tensor.matmul(
                                        po[w * il_lo:w * (il_lo + 1), jc, t, :],
                                        lhsT=wcols, rhs=v_bf[:, kc, :],
                                        start=(kc == kcs[0]), stop=(kc == kcs[-1]),
                                        tile_position=(0, w * il_lo))
                    nc.vector.tensor_copy(out=ost[:, g], in_=po)
            else:
                ost = outp.tile([128, IB, JC, P], F32, tag="ost")
                if "no_mm" in ablate:
                    nc.vector.memset(ost[:, 0, 0, 0:2], 0.0)
                else:
                    for g in range(IB // GRP):
                        po = pso.tile([128, GRP, JC, P], F32, tag="po")
                        for gi in range(GRP):
                            il = g * GRP + gi
                            for jc in range(JC):
                                for kc in kcs:
                                    nc.tensor.matmul(po[:, gi, jc, :],
                                                     lhsT=src[:, kc, il, jc * 128:(jc + 1) * 128],
                                                     rhs=v_bf[:, kc, :],
                                                     start=(kc == kcs[0]), stop=(kc == kcs[-1]))
                        nc.vector.tensor_copy(out=ost[:, g * GRP:(g + 1) * GRP], in_=po)
                if "gather" in ablate:
                    # SBUF->SBUF partition regroup (small descriptors are cheap
                    # off-HBM), so the HBM store runs with 6.4KB/partition runs.
                    lin = outp.tile([128, 32, P], F32, tag="lin")
                    for il in range(IB):
                        for jc in range(JC):
                            nc.sync.dma_start(
                                out=lin[8 * il + 4 * jc: 8 * il + 4 * jc + 4],
                                in_=ost[:, il, jc, :])
            if "no_dma" in ablate:
                pass
            elif "lin_dma" in ablate:
                nc.sync.dma_start(out=lin_scr[blk % (S // IB)], in_=ost)
            elif "relay" in ablate or "relay2" in ablate:
                # two-hop store: linear dump (line-rate writes), then a
                # DRAM->DRAM relayout whose writes are linear and whose reads
                # are 200B-strided (reads don't pay the sub-512B RMW penalty)
                b = blk % (S // IB)
                nc.sync.dma_start(out=lin_scr[b], in_=ost)
                eng = nc.scalar if "relay2" in ablate else nc.sync
                eng.dma_start(
                    out=sc[blk * IB:(blk + 1) * IB].rearrange("i (jc jl) p -> i jc jl p", jc=JC),
                    in_=lin_scr[b].rearrange("jl i jc p -> i jc jl p"),
                )
            elif "gather" in ablate:
                oap = sc[blk * IB:(blk + 1) * IB].rearrange(
                    "i (jc m jlo) p -> (i jc m) jlo p", jc=JC, m=4)
                nc.sync.dma_start(out=oap, in_=lin)
            elif nt:
                # ost[part=w*il_lo+c, g, jc, t, p] = scores[blk*IB+g*ng+il_lo,
                #                                           jc*128+nt*c+t, p]
                w = 128 // nt
                ng = 128 // w
                for g in range(IB // ng):
                    i0 = blk * IB + g * ng
                    oap = sc[i0:i0 + ng].rearrange(
                        "il_lo (jc c t) p -> il_lo c jc (t p)", jc=JC, t=nt)
                    for jc in range(JC):
                        nc.sync.dma_start(out=oap[:, :, jc, :], in_=ost[:, g, jc],
                                          single_packet="pkt" in ablate)
            else:
                oap = sc[blk * IB:(blk + 1) * IB].rearrange("i (jc jl) p -> jl i jc p", jc=JC)
                eng = nc.scalar if ("dma_split" in ablate and blk % 2) else nc.sync
                eng.dma_start(out=oap, in_=ost, single_packet="pkt" in ablate)

    return nc


_RUNNERS = {}


def _get_runner(reps=1, ablate=()):
    key = (reps, tuple(sorted(ablate)))
    if key in _RUNNERS:
        return _RUNNERS[key]
    import jax
    from jax.sharding import Mesh, PartitionSpec
    from jax.experimental.shard_map import shard_map
    from concourse.bass2jax import install_neuronx_cc_hook, _bass_exec_p

    install_neuronx_cc_hook()
    nc = _build_nc(reps=reps, ablate=ablate)
    if not nc.is_finalized():
        nc.finalize()

    in_names, out_names, out_avals = [], [], []
    for alloc in nc.m.functions[0].allocations:
        if not isinstance(alloc, mybir.MemoryLocationSet):
            continue
        if alloc.kind not in ("ExternalInput", "ExternalOutput"):
            continue
        name = alloc.memorylocations[0].name
        if alloc.kind == "ExternalInput":
            in_names.append(name)
        else:
            out_names.append(name)
            out_avals.append(jax.core.ShapedArray(tuple(alloc.tensor_shape),
                                                  mybir.dt.np(alloc.dtype)))
    n_params = len(in_names)
    all_in_names = tuple(in_names + out_names)

    def _body(*args):
        outs = _bass_exec_p.bind(
            *args,
            out_avals=tuple(out_avals),
            in_names=all_in_names,
            out_names=tuple(out_names),
            lowering_input_output_aliases=(),
            sim_require_finite=True,
            sim_require_nnan=True,
            nc=nc,
        )
        return tuple(outs)

    devices = jax.devices()[:NCORES]
    assert len(devices) == NCORES, f"need {NCORES} cores, got {len(devices)}"
    mesh = Mesh(np.asarray(devices), ("core",))
    nin = n_params + len(out_names)
    fn = jax.jit(
        shard_map(_body, mesh=mesh,
                  in_specs=(PartitionSpec("core"),) * nin,
                  out_specs=(PartitionSpec("core"),) * len(out_names),
                  check_rep=False),
        keep_unused=True,
    )
    _RUNNERS[key] = (fn, in_names, out_names, out_avals, mesh)
    return _RUNNERS[key]


def _concat_args(x, u_a, w_a, b_s, v, in_names, out_avals):
    x = np.ascontiguousarray(np.asarray(x, dtype=np.float32))
    u_a = np.asarray(u_a, dtype=np.float32)
    w_a = np.asarray(w_a, dtype=np.float32)
    b_s = np.asarray(b_s, dtype=np.float32)
    v = np.asarray(v, dtype=np.float32)
    per = {
        "xb": x.reshape(NCORES * S, H),
        "ua": np.tile(u_a, (NCORES, 1)),
        "wa": np.tile(w_a, (NCORES, 1)),
        "bs": np.tile(b_s, NCORES),
        "vv": np.tile(v, (NCORES, 1)),
    }
    args = [per[n] for n in in_names]
    args += [np.zeros((NCORES * a.shape[0], *a.shape[1:]), a.dtype) for a in out_avals]
    return args


def kernel(x, u_a, w_a, b_s, v):
    fn, in_names, out_names, out_avals, mesh = _get_runner()
    args = _concat_args(x, u_a, w_a, b_s, v, in_names, out_avals)
    outs = fn(*args)
    scores = np.asarray(outs[out_names.index("scores")])
    return scores.reshape(B, S, S, P)


def _timed_calls(reps, x, u_a, w_a, b_s, v, iters, ablate=()):
    import time
    import jax
    from jax.sharding import NamedSharding, PartitionSpec

    fn, in_names, out_names, out_avals, mesh = _get_runner(reps=reps, ablate=ablate)
    args = _concat_args(x, u_a, w_a, b_s, v, in_names, out_avals)
    sh = NamedSharding(mesh, PartitionSpec("core"))
    dargs = [jax.device_put(a, sh) for a in args]
    for _ in range(3):  # warmup (also triggers compile)
        outs = fn(*dargs)
    jax.block_until_ready(outs)
    times = []
    for _ in range(iters):
        t0 = time.perf_counter()
        out = fn(*dargs)
        jax.block_until_ready(out)
        times.append(time.perf_counter() - t0)
    return times


def bench(x, u_a, w_a, b_s, v, iters=10, r_hi=5):
    """Estimate on-device time of one full computation.

    Runs NEFFs with the stage-2 loop executed once and r_hi times; the
    difference isolates device time from per-call host/axon dispatch
    overhead. Returns seconds for one computation (stage2 delta-based).
    """
    t1 = _timed_calls(1, x, u_a, w_a, b_s, v, iters)
    th = _timed_calls(r_hi, x, u_a, w_a, b_s, v, iters)
    t1m, thm = min(t1), min(th)
    stage2 = (thm - t1m) / (r_hi - 1)
    return stage2, dict(t_r1=t1m, t_rhi=thm, r_hi=r_hi,
                        med_r1=sorted(t1)[len(t1) // 2],
                        med_rhi=sorted(th)[len(th) // 2])



# revision 2
# speedup vs baseline: 1.1660x; 1.1660x over previous
"""MultiHeadSelection Trainium2 kernel (i-on-partition output layout).

scores[b,i,j,p] = sum_k tanh(x[b,i]@u_a[:,k] + x[b,j]@w_a[:,k] + b_s[k]) * v[k,p]

Shapes (hardcoded): x [8,256,768], u_a/w_a [768,256], b_s [256], v [256,50]
-> out [8,256,256,50] float32.

Sharding: data-parallel over batch, one batch element per NeuronCore (8 cores).

Per core (swapped-role formulation):
  stage 1: l_T[k,i] = (x_b @ u_a)^T  (bf16, the tensor operand of the preadd)
           rb[k,j]  = (x_b @ w_a)^T + b_s  (fp32, the per-(k,j) scalar)
  stage 2: loop over j-blocks (JB j's):
           pre[k,(jl,i)] = l_T[k,i] + rb[k,j]   (tensor_scalar ptr, DVE/GPSIMD)
           th = tanh(pre)                        (ACT, FD=JB*S per op)
           psum[i_chunk, (jg,p)] += th[k,jl,ic]^T @ v[k,p]  (PE, 2 kc)
           DVE copy psum -> ost[ic][128 i, SB j, P] staging
           per SB j's: one DMA per ic: scores[ic*128+:128, j0:j0+SB, :]
           (per-partition runs of SB*P*4 = 12.8KB -> line-rate HBM store)
"""

import numpy as np
from contextlib import ExitStack

import concourse.bass as bass
import concourse.mybir as mybir
import concourse.tile as tile
from concourse import bacc

B, S, H, K, P = 8, 256, 768, 256, 50
NCORES = 8
JB = 16            # j-block size (ACT op free dim = JB*S = 4096)
JG = 8             # j's per psum tile ([128, JG*P] = 1600B/bank)
SB = 64            # j's per output staging tile / DMA (12.8KB per partition)
KC = K // 128      # 2 k-chunks
HC = H // 128      # 6 h-chunks
IC = S // 128      # 2 i-chunks (psum/output partition dim)
ACT_COPIES = 1     # of the 4 psum->SBUF copies per block, how many on ScalarE

F32 = mybir.dt.float32
BF16 = mybir.dt.bfloat16


def _build_nc(reps=1, ablate=()):
    ablate = set(ablate)
    act_copies = 0 if "no_actcp" in ablate else ACT_COPIES
    # Bacc (not raw Bass): its compile() pass splits multi-semaphore waits
    # into EventSemaphore instructions — TRN2 engine instructions hold 1 wait.
    nc = bacc.Bacc("TRN2", target_bir_lowering=False, debug=False,
                   enable_partition_id=False)

    xb = nc.dram_tensor("xb", [S, H], F32, kind="ExternalInput").ap()
    ua = nc.dram_tensor("ua", [H, K], F32, kind="ExternalInput").ap()
    wa = nc.dram_tensor("wa", [H, K], F32, kind="ExternalInput").ap()
    bs = nc.dram_tensor("bs", [K], F32, kind="ExternalInput").ap()
    vv = nc.dram_tensor("vv", [K, P], F32, kind="ExternalInput").ap()
    sc = nc.dram_tensor("scores", [S, S, P], F32, kind="ExternalOutput").ap()

    with ExitStack() as ctx:
        tc = ctx.enter_context(tile.TileContext(nc))
        singles = ctx.enter_context(tc.tile_pool(name="singles", bufs=1))
        work = ctx.enter_context(tc.tile_pool(name="work", bufs=2))
        outp = ctx.enter_context(tc.tile_pool(name="outp", bufs=2))

        # ---- constants ----
        v_bf = singles.tile([128, KC, P], BF16)
        for kc in range(KC):
            nc.gpsimd.dma_start(out=v_bf[:, kc, :], in_=vv[kc * 128:(kc + 1) * 128, :])
        bs_dma = singles.tile([128, KC], F32)
        for kc in range(KC):
            nc.sync.dma_start(out=bs_dma[:, kc:kc + 1], in_=bs[kc * 128:(kc + 1) * 128])
        # Bounce through a DVE copy so the DMA-completion wait lands on the
        # copy, not on the single-wait-slot TensorScalarPtr that consumes it.
        bs_col = singles.tile([128, KC], F32)
        nc.vector.tensor_copy(out=bs_col, in_=bs_dma)

        l_bf = singles.tile([128, KC, S], BF16)   # left_T, bf16 (preadd in0)
        rb = singles.tile([128, KC, S], F32)      # right_T + b_s (preadd scalar)

        # ---- stage 1 ----
        from concourse.masks import make_identity
        with tc.tile_pool(name="s1", bufs=1) as s1, \
             tc.tile_pool(name="ps1", bufs=2, space="PSUM") as ps1:
            # x: HWDGE f32 load + DVE cast + PE transposes. DMA-transposes are
            # serialized ~4us apiece by the deadlock guard; the PE does a
            # 128x128 transpose in ~0.4us.
            x_f = s1.tile([128, IC, H], F32)
            for icc in range(IC):
                nc.sync.dma_start(out=x_f[:, icc, :], in_=xb[icc * 128:(icc + 1) * 128, :])
            x_bf = s1.tile([128, IC, H], BF16)
            nc.vector.tensor_copy(out=x_bf, in_=x_f)
            ident = s1.tile([128, 128], BF16)
            make_identity(nc, ident)
            x_T = s1.tile([128, HC, S], BF16)
            for hc in range(HC):
                for icc in range(IC):
                    pst = ps1.tile([128, 128], BF16, tag="pst")
                    nc.tensor.transpose(pst, in_=x_bf[:, icc, hc * 128:(hc + 1) * 128],
                                        identity=ident)
                    nc.vector.tensor_copy(out=x_T[:, hc, icc * 128:(icc + 1) * 128],
                                          in_=pst)

            # u/w: HWDGE f32 loads + DVE casts (the lone SWDGE queue stays free)
            u_f = s1.tile([128, HC, K], F32)
            w_f = s1.tile([128, HC, K], F32)
            for hc in range(HC):
                nc.sync.dma_start(out=u_f[:, hc, :], in_=ua[hc * 128:(hc + 1) * 128, :])
                nc.sync.dma_start(out=w_f[:, hc, :], in_=wa[hc * 128:(hc + 1) * 128, :])
            u_bf = s1.tile([128, HC, K], BF16)
            w_bf = s1.tile([128, HC, K], BF16)
            for hc in range(HC):
                nc.vector.tensor_copy(out=u_bf[:, hc, :], in_=u_f[:, hc, :])
                nc.vector.tensor_copy(out=w_bf[:, hc, :], in_=w_f[:, hc, :])

            for kc in range(KC):
                ps_l = ps1.tile([128, S], F32, tag="ps_l")
                ps_r = ps1.tile([128, S], F32, tag="ps_r")
                for hc in range(HC):
                    nc.tensor.matmul(ps_l, lhsT=u_bf[:, hc, kc * 128:(kc + 1) * 128],
                                     rhs=x_T[:, hc, :], start=(hc == 0), stop=(hc == HC - 1))
                for hc in range(HC):
                    nc.tensor.matmul(ps_r, lhsT=w_bf[:, hc, kc * 128:(kc + 1) * 128],
                                     rhs=x_T[:, hc, :], start=(hc == 0), stop=(hc == HC - 1))
                nc.vector.tensor_copy(out=l_bf[:, kc, :], in_=ps_l)
                # Two-step (copy then add) keeps the TensorScalarPtr at a
                # single semaphore wait: its ISA encoding has only one wait
                # slot, and a direct PSUM read would need PE + DMA waits.
                rt = s1.tile([128, S], F32, tag="rt")
                nc.vector.tensor_copy(out=rt, in_=ps_r)
                nc.vector.tensor_scalar_add(out=rb[:, kc, :], in0=rt,
                                            scalar1=bs_col[:, kc:kc + 1])

        # ---- stage 2 ----
        pso = ctx.enter_context(tc.tile_pool(name="pso", bufs=8, space="PSUM"))
        nblk = S // JB
        nsb = SB // JB  # blocks per output superblock
        ost_dt = F32 if "f32out" in ablate else BF16
        ost = [None, None]
        pending = []    # deferred psum->staging copies: (po, dst, engine)
        old_dma = None  # deferred superblock store

        def flush():
            # Copies for the PREVIOUS block are emitted here, AFTER the current
            # block's preadds+tanh sit in the engine queues: by the time each
            # engine reaches them the previous block's matmuls are done, so the
            # copies never stall the DVE/ACT FIFOs mid-pipeline.
            nonlocal old_dma
            for po_, dst_, eng_ in pending:
                if eng_ == "act":
                    nc.scalar.copy(out=dst_, in_=po_)
                else:
                    nc.vector.tensor_copy(out=dst_, in_=po_)
            pending.clear()
            if old_dma is not None:
                jsb_, ost_ = old_dma
                for ic in range(IC):
                    if ost_dt is F32:
                        nc.sync.dma_start(out=sc[ic * 128:(ic + 1) * 128, jsb_:jsb_ + SB, :],
                                          in_=ost_[ic])
                    else:
                        # bf16 staging -> fp32 HBM: SWDGE casts during the store
                        nc.gpsimd.dma_start(out=sc[ic * 128:(ic + 1) * 128, jsb_:jsb_ + SB, :],
                                            in_=ost_[ic])
                old_dma = None

        total_iters = reps * nblk
        for it, blk in enumerate(b for _ in range(reps) for b in range(nblk)):
            last_sb = it >= total_iters - nsb  # final superblock: no deferral
            j0 = (blk % nblk) * JB
            pre = work.tile([128, KC, JB, S], BF16, tag="pre")
            th = work.tile([128, KC, JB, S], BF16, tag="th")
            # Absorb the buffer-reuse (WAR vs ACT) semaphore waits into this
            # memset: the TensorScalarPtr ISA struct has only one sync-wait
            # slot, so the preadds below must not carry cross-engine waits.
            nc.vector.memset(pre[:, 0, 0, 0:2], 0.0)
            for kc in range(KC):
                for jl in range(JB):
                    nc.vector.tensor_scalar_add(out=pre[:, kc, jl, :],
                                                in0=l_bf[:, kc, :],
                                                scalar1=rb[:, kc, j0 + jl:j0 + jl + 1])
                if it == 0:
                    # First block: split the tanh so ACT starts after 4 preadds
                    # instead of 16 — shortens the serial stage-1 ramp.
                    for q in range(4):
                        nc.scalar.activation(out=th[:, kc, 4 * q:4 * (q + 1)],
                                             in_=pre[:, kc, 4 * q:4 * (q + 1)],
                                             func=mybir.ActivationFunctionType.Tanh)
                else:
                    nc.scalar.activation(out=th[:, kc], in_=pre[:, kc],
                                         func=mybir.ActivationFunctionType.Tanh)
            flush()

            if blk % nsb == 0:
                ost = [outp.tile([128, SB, P], ost_dt, tag=f"ost{ic}",
                                 name=f"ost{ic}") for ic in range(IC)]
            jo = (blk % nsb) * JB  # offset of this block inside ost
            ncp = 0
            for g in range(JB // JG):
                for ic in range(IC):
                    po = pso.tile([128, JG, P], F32, tag="po")
                    for jg in range(JG):
                        jl = g * JG + jg
                        for kc in range(KC):
                            nc.tensor.matmul(po[:, jg, :],
                                             lhsT=th[:, kc, jl, ic * 128:(ic + 1) * 128],
                                             rhs=v_bf[:, kc, :],
                                             start=(kc == 0), stop=(kc == KC - 1))
                    dst = ost[ic][:, jo + g * JG:jo + g * JG + JG]
                    if last_sb:
                        # Tail: copy + store per psum group immediately (1.6KB
                        # runs are still line-rate) so the end doesn't
                        # serialize a whole deferred superblock after the
                        # final tanh.
                        nc.vector.tensor_copy(out=dst, in_=po)
                        nc.gpsimd.dma_start(
                            out=sc[ic * 128:(ic + 1) * 128,
                                   j0 + g * JG:j0 + (g + 1) * JG, :],
                            in_=dst)
                    else:
                        pending.append((po, dst, "act" if ncp < act_copies else "dve"))
                    ncp += 1
            if (not last_sb) and blk % nsb == nsb - 1:
                old_dma = (j0 + JB - SB, ost)
        flush()

    return nc


_RUNNERS = {}


def _get_runner(reps=1, ablate=()):
    key = (reps, tuple(sorted(ablate)))
    if key in _RUNNERS:
        return _RUNNERS[key]
    import jax
    from jax.sharding import Mesh, PartitionSpec
    from jax.experimental.shard_map import shard_map
    from concourse.bass2jax import install_neuronx_cc_hook, _bass_exec_p

    install_neuronx_cc_hook()
    nc = _build_nc(reps=reps, ablate=ablate)
    if not nc.is_finalized():
        nc.finalize()

    in_names, out_names, out_avals = [], [], []
    for alloc in nc.m.functions[0].allocations:
        if not isinstance(alloc, mybir.MemoryLocationSet):
            continue
        if alloc.kind not in ("ExternalInput", "ExternalOutput"):
            continue
        name = alloc.memorylocations[0].name
        if alloc.kind == "ExternalInput":
            in_names.append(name)
        else:
            out_names.append(name)
            out_avals.append(jax.core.ShapedArray(tuple(alloc.tensor_shape),
                                                  mybir.dt.np(alloc.dtype)))
    n_params = len(in_names)
    all_in_names = tuple(in_names + out_names)

    def _body(*args):
        outs = _bass_exec_p.bind(
            *args,
            out_avals=tuple(out_avals),
            in_names=all_in_names,
            out_names=tuple(out_names),
            lowering_input_output_aliases=(),
            sim_require_finite=True,
            sim_require_nnan=True,
            nc=nc,
        )
        return tuple(outs)

    devices = jax.devices()[:NCORES]
    assert len(devices) == NCORES, f"need {NCORES} cores, got {len(devices)}"
    mesh = Mesh(np.asarray(devices), ("core",))
    nin = n_params + len(out_names)
    fn = jax.jit(
        shard_map(_body, mesh=mesh,
                  in_specs=(PartitionSpec("core"),) * nin,
                  out_specs=(PartitionSpec("core"),) * len(out_names),
                  check_rep=False),
        keep_unused=True,
    )
    _RUNNERS[key] = (fn, in_names, out_names, out_avals, mesh)
    return _RUNNERS[key]


def _concat_args(x, u_a, w_a, b_s, v, in_names, out_avals):
    x = np.ascontiguousarray(np.asarray(x, dtype=np.float32))
    u_a = np.asarray(u_a, dtype=np.float32)
    w_a = np.asarray(w_a, dtype=np.float32)
    b_s = np.asarray(b_s, dtype=np.float32)
    v = np.asarray(v, dtype=np.float32)
    per = {
        "xb": x.reshape(NCORES * S, H),
        "ua": np.tile(u_a, (NCORES, 1)),
        "wa": np.tile(w_a, (NCORES, 1)),
        "bs": np.tile(b_s, NCORES),
        "vv": np.tile(v, (NCORES, 1)),
    }
    args = [per[n] for n in in_names]
    args += [np.zeros((NCORES * a.shape[0], *a.shape[1:]), a.dtype) for a in out_avals]
    return args


def kernel(x, u_a, w_a, b_s, v):
    fn, in_names, out_names, out_avals, mesh = _get_runner()
    args = _concat_args(x, u_a, w_a, b_s, v, in_names, out_avals)
    outs = fn(*args)
    scores = np.asarray(outs[out_names.index("scores")])
    return scores.reshape(B, S, S, P)


def _timed_calls(reps, x, u_a, w_a, b_s, v, iters, ablate=()):
    import time
    import jax
    from jax.sharding import NamedSharding, PartitionSpec

    fn, in_names, out_names, out_avals, mesh = _get_runner(reps=reps, ablate=ablate)
    args = _concat_args(x, u_a, w_a, b_s, v, in_names, out_avals)
    sh = NamedSharding(mesh, PartitionSpec("core"))
    dargs = [jax.device_put(a, sh) for a in args]
    for _ in range(3):  # warmup (also triggers compile)
        outs = fn(*dargs)
    jax.block_until_ready(outs)
    times = []
    for _ in range(iters):
        t0 = time.perf_counter()
        out = fn(*dargs)
        jax.block_until_ready(out)
        times.append(time.perf_counter() - t0)
    return times


def bench(x, u_a, w_a, b_s, v, iters=10, r_hi=5):
    """Wall-clock fallback: run NEFFs with the stage-2 loop executed once and
    r_hi times; the difference estimates device time net of host dispatch.
    NOISY under axon — prefer profile_hw()."""
    t1 = _timed_calls(1, x, u_a, w_a, b_s, v, iters)
    th = _timed_calls(r_hi, x, u_a, w_a, b_s, v, iters)
    t1m, thm = min(t1), min(th)
    stage2 = (thm - t1m) / (r_hi - 1)
    return stage2, dict(t_r1=t1m, t_rhi=thm, r_hi=r_hi)


def profile_hw(reps=1, ablate=(), outdir=None, trace_cores=(0,), convert=True,
               trials=1):
    """Run the kernel NEFF under the axon NTFF profile hook; return
    (exec_time_ns_by_core, outdir). Real random inputs, all 8 cores running.
    With trials>1, captures/converts each trial and reports the per-core
    MEDIAN across trials (device-state noise across runs is +/-15%)."""
    if trials > 1:
        alltimes = []
        for t in range(trials):
            times, outdir_t = profile_hw(reps=reps, ablate=ablate,
                                         outdir=f"{outdir or '/root/problem/traces/hw'}_t{t}",
                                         trace_cores=trace_cores, convert=convert)
            alltimes.append(times)
        med = {}
        for c in alltimes[0]:
            vals = sorted(t[c] for t in alltimes if c in t)
            med[c] = vals[len(vals) // 2]
        print("profile trials:", alltimes)
        return med, f"{outdir or '/root/problem/traces/hw'}_t*"
    import ctypes
    import glob
    import os
    import shutil
    import subprocess
    import json
    import jax
    from jax.sharding import NamedSharding, PartitionSpec

    fn, in_names, out_names, out_avals, mesh = _get_runner(reps=reps, ablate=ablate)
    rng = np.random.default_rng(0)
    x = rng.standard_normal((B, S, H), dtype=np.float32)
    u_a = (rng.standard_normal((H, K)) * 0.02).astype(np.float32)
    w_a = (rng.standard_normal((H, K)) * 0.02).astype(np.float32)
    b_s = (rng.standard_normal((K,)) * 0.02).astype(np.float32)
    v = (rng.standard_normal((K, P)) * 0.02).astype(np.float32)
    args = _concat_args(x, u_a, w_a, b_s, v, in_names, out_avals)
    sh = NamedSharding(mesh, PartitionSpec("core"))
    dargs = [jax.device_put(a, sh) for a in args]
    for _ in range(2):
        outs = fn(*dargs)
    jax.block_until_ready(outs)

    if outdir is None:
        outdir = f"/root/problem/traces/hw_r{reps}_{'_'.join(sorted(ablate)) or 'base'}"
    shutil.rmtree(outdir, ignore_errors=True)
    os.makedirs(outdir, exist_ok=True)

    lib = ctypes.CDLL("/opt/axon/libaxon_pjrt.so")
    lib.axon_start_nrt_profile.argtypes = [ctypes.POINTER(ctypes.c_int64), ctypes.c_size_t]
    lib.axon_start_nrt_profile.restype = ctypes.c_int64
    lib.axon_stop_nrt_profile.argtypes = [ctypes.c_char_p]
    lib.axon_stop_nrt_profile.restype = ctypes.c_int64
    ids = (ctypes.c_int64 * len(trace_cores))(*trace_cores)
    rc = lib.axon_start_nrt_profile(ids, len(trace_cores))
    assert rc == 0, f"axon_start_nrt_profile rc={rc}"
    outs = fn(*dargs)
    jax.block_until_ready(outs)
    nfiles = lib.axon_stop_nrt_profile(outdir.encode())
    assert nfiles > 0, "profile capture produced no files"

    if not convert:
        return {}, outdir

    neffs = sorted(glob.glob(f"{outdir}/*.neff"))
    assert neffs, f"no NEFF shipped back to {outdir}"
    times = {}
    procs = []
    for ntff in sorted(glob.glob(f"{outdir}/*.ntff")):
        dev = ntff.split("device")[1][:6]
        jout = f"{outdir}/ntff_{dev}.json"
        procs.append((dev, jout, subprocess.Popen(
            ["neuron-profile", "view", "-n", neffs[0], "-s", ntff,
             "--output-format=json", "--output-file", jout,
             "--ignore-nc-buf-usage"],
            stdout=subprocess.DEVNULL, stderr=subprocess.DEVNULL)))
    for dev, jout, pr in procs:
        pr.wait()
        if os.path.exists(jout):
            d = json.load(open(jout))
            s = d["summary"][0] if isinstance(d["summary"], list) else d["summary"]
            times[int(dev)] = int(s["total_time"] * 1e9)
    return times, outdir
